# revision 6
# baseline (speedup 1.0000x reference)
"""BiLSTM layer (T=1024, B=64, D=128, H=256) on 8 TRN2 NeuronCores.

Sharding: direction x batch. Cores 0-3 run the forward LSTM, cores 4-7 the
reverse LSTM (fed time-flipped inputs so the compiled SPMD program is
identical on every core); each core owns a 16-batch slice.

Per-core layout ("transposed" gates): gates/h/c live as [feature-partition,
batch-free] tiles, so the per-step recurrent matmul is 16 LDWEIGHTS+MATMUL
pairs (8 gate tiles x 2 K-tiles) with Whh stationary in fp16 (fast weight
load) and hT as a 16-column moving operand. The input projection
gx = x @ Wih.T + b is computed on-device in 32-step chunks, double-buffered
and interleaved with the recurrence so it hides in PE gaps.

Gate order is host-permuted from torch's (i,f,g,o) to (g,i,f,o) so tanh gates
occupy one contiguous column block and sigmoid gates another.
"""

import numpy as np

import concourse.bass as bass
import concourse.mybir as mybir
from concourse.tile import TileContext

T, B, D, H = 1024, 64, 128, 256
G4 = 4 * H  # 1024
NCORES = 8
BL = B // 4  # 16 batches per core (4 cores per direction)
TC = 32  # time-steps per sub-chunk (=> 512 x-columns per gx matmul)
NSUB = T // TC  # 32 sub-chunks
F32 = mybir.dt.float32

# fp16 for the matmul operands (weights, h, x): fast weight load + 11-bit
# mantissa; accumulation stays fp32 in PSUM and all state/outputs are fp32.
RNN_DT = mybir.dt.float16
RNN_NP = np.float16

# row permutation taking torch gate order (i,f,g,o) -> (g,i,f,o)
_PERM = np.concatenate(
    [np.arange(512, 768), np.arange(0, 256), np.arange(256, 512), np.arange(768, 1024)]
)
# column offsets (units of BL) of each gate in the packed [128, 8*16] tile
_GC, _IC, _FC, _OC = 0, 2, 4, 6


def _split_excess_waits(nc, max_waits: int = 1):
    """This walrus build rejects instructions carrying more than one sync
    wait. Hoist excess waits onto same-engine nop instructions inserted
    immediately before the offending instruction (same program order on that
    engine's stream, so semantics are preserved)."""
    fn = nc.m.functions[0]
    # pass 1: find spills
    spills = []  # (bb_idx, inst_name, engine, [extra waits])
    for bi, bb in enumerate(fn.blocks):
        for ins in bb.instructions:
            si = ins.sync_info
            if si is None:
                continue
            waits = list(si.on_wait)
            if len(waits) > max_waits:
                spills.append((bi, str(ins.name), ins.engine, waits))
    if not spills:
        return
    # pass 2: create the nop instructions (they land appended to the current
    # tail block; pass 3 relocates them)
    nop_names = set()
    nops_for = {}
    for bi, iname, engine, waits in spills:
        nops = []
        for w in waits[max_waits:]:
            nop = nc.engines[engine].nop(nofuse=True)
            nop.ins.sync_info = mybir.SyncInfo(on_wait=[w], on_update=[])
            nop_names.add(str(nop.ins.name))
            nops.append(nop.ins)
        nops_for[(bi, iname)] = nops
    # pass 3: rebuild blocks with nops in position (and stripped from tail)
    new_blocks = []
    for bi, bb in enumerate(fn.blocks):
        out = []
        for ins in bb.instructions:
            nm = str(ins.name)
            if nm in nop_names:
                continue  # relocated
            pre = nops_for.get((bi, nm))
            if pre is not None:
                si = ins.sync_info
                ins.sync_info = mybir.SyncInfo(
                    on_wait=list(si.on_wait)[:max_waits],
                    on_update=list(si.on_update),
                )
                out.extend(pre)
            out.append(ins)
        new_blocks.append(
            mybir.BasicBlock(
                name=bb.name,
                instructions=out,
                IsPredicated=bb.IsPredicated,
                IsExit=bb.IsExit,
                IsLoopEntry=bb.IsLoopEntry,
            )
        )
    fn.blocks = new_blocks


def build_nc() -> bass.Bass:
    nc = bass.Bass()

    xt = nc.declare_dram_parameter("xt", [128, (T + TC) * BL], RNN_DT, isOutput=False)
    whh = nc.declare_dram_parameter("whh", [128, 2 * G4], RNN_DT, isOutput=False)
    wih = nc.declare_dram_parameter("wih", [128, G4], RNN_DT, isOutput=False)
    bias = nc.declare_dram_parameter("bias", [128, 8], F32, isOutput=False)
    h0 = nc.declare_dram_parameter("h0", [128, 2 * BL], F32, isOutput=False)
    c0 = nc.declare_dram_parameter("c0", [128, 2 * BL], F32, isOutput=False)
    yt = nc.declare_dram_parameter("yt", [128, T * 2 * BL], F32, isOutput=True)
    hc = nc.declare_dram_parameter("hc", [128, 4 * BL], F32, isOutput=True)

    W = 2 * BL  # 32: width of h/c tiles (2 K-tiles side by side)
    SIG = mybir.ActivationFunctionType.Sigmoid
    TANH = mybir.ActivationFunctionType.Tanh
    IDENT = mybir.ActivationFunctionType.Identity

    with TileContext(nc) as tc:
        with (
            tc.tile_pool(name="persist", bufs=1) as pp,
            tc.tile_pool(name="work", bufs=3) as wp,
            tc.tile_pool(name="pg", bufs=2, space="PSUM") as pgp,
            tc.tile_pool(name="pgx", bufs=3, space="PSUM") as pgxp,
        ):
            # ---- persistent tiles ----
            whh_sb = pp.tile([128, 2 * G4], RNN_DT, tag="whh_sb")
            wih_sb = pp.tile([128, G4], RNN_DT, tag="wih_sb")
            bias_sb = pp.tile([128, 8], F32, tag="bias_sb")
            h16 = pp.tile([128, W], RNN_DT, tag="h16")
            c32 = pp.tile([128, W], F32, tag="c32")
            h0f = pp.tile([128, W], F32, tag="h0f")
            xc = [pp.tile([128, TC * BL], RNN_DT, tag=f"xc{j}", name=f"xc{j}") for j in range(2)]
            gxb = [pp.tile([128, 8, TC * BL], F32, tag=f"gxb{j}", name=f"gxb{j}") for j in range(2)]
            yac = [pp.tile([128, TC, W], F32, tag=f"yac{j}", name=f"yac{j}") for j in range(2)]

            nc.sync.dma_start(out=whh_sb[:], in_=whh[:])
            nc.sync.dma_start(out=wih_sb[:], in_=wih[:])
            nc.sync.dma_start(out=bias_sb[:], in_=bias[:])
            nc.sync.dma_start(out=h0f[:], in_=h0[:])
            nc.sync.dma_start(out=c32[:], in_=c0[:])
            nc.vector.tensor_copy(h16[:], h0f[:])

            def gx_chunk(j: int, col0):
                """Project x columns [col0, col0+TC*BL) into gxb[j] (+bias)."""
                nc.sync.dma_start(out=xc[j][:], in_=xt[:, bass.ds(col0, TC * BL)])
                for m in range(8):
                    pgx = pgxp.tile([128, TC * BL], F32, tag="pgx")
                    nc.tensor.matmul(
                        pgx[:],
                        wih_sb[:, m * 128 : (m + 1) * 128],
                        xc[j][:],
                        start=True,
                        stop=True,
                    )
                    nc.scalar.activation(
                        gxb[j][:, m, :], pgx[:], IDENT, bias=bias_sb[:, m : m + 1]
                    )

            def step(j: int, s: int):
                """One LSTM time-step; gates from gxb[j][:, :, s*BL:(s+1)*BL]."""
                pg = pgp.tile([128, 8 * BL], F32, tag="pg")
                pg3 = pg.rearrange("p (m b) -> p m b", b=BL)
                for m in range(8):
                    for k in range(2):
                        nc.tensor.matmul(
                            pg[:, m * BL : (m + 1) * BL],
                            whh_sb[:, k * G4 + m * 128 : k * G4 + (m + 1) * 128],
                            h16[:, k * BL : (k + 1) * BL],
                            start=(k == 0),
                            stop=(k == 1),
                        )
                gt = wp.tile([128, 8 * BL], F32, tag="gt")
                gt3 = gt.rearrange("p (m b) -> p m b", b=BL)
                gxs = gxb[j][:, :, s * BL : (s + 1) * BL]
                # gates = psum + gx, split so the first add runs while the
                # o-gate matmuls are still streaming
                nc.vector.tensor_add(gt3[:, 0:6, :], pg3[:, 0:6, :], gxs[:, 0:6, :])
                nc.vector.tensor_add(gt3[:, 6:8, :], pg3[:, 6:8, :], gxs[:, 6:8, :])
                ac = wp.tile([128, 8 * BL], F32, tag="ac")
                g_, i_, f_, o_ = (
                    slice(_GC * BL, (_GC + 2) * BL),
                    slice(_IC * BL, (_IC + 2) * BL),
                    slice(_FC * BL, (_FC + 2) * BL),
                    slice(_OC * BL, (_OC + 2) * BL),
                )
                nc.scalar.activation(ac[:, g_], gt[:, g_], TANH)
                nc.scalar.activation(
                    ac[:, _IC * BL : (_FC + 2) * BL], gt[:, _IC * BL : (_FC + 2) * BL], SIG
                )
                nc.scalar.activation(ac[:, o_], gt[:, o_], SIG)
                t1 = wp.tile([128, W], F32, tag="t1")
                t2 = wp.tile([128, W], F32, tag="t2")
                nc.vector.tensor_mul(t1[:], ac[:, i_], ac[:, g_])
                nc.vector.tensor_mul(t2[:], ac[:, f_], c32[:])
                nc.vector.tensor_add(c32[:], t1[:], t2[:])
                tcc = wp.tile([128, W], F32, tag="tcc")
                nc.scalar.activation(tcc[:], c32[:], TANH)
                nc.vector.tensor_mul(yac[j][:, s, :], ac[:, o_], tcc[:])
                nc.vector.tensor_mul(h16[:], ac[:, o_], tcc[:])

            # prologue: gx for sub-chunk 0 into buffer 0
            gx_chunk(0, 0)

            with tc.For_i(
                0, NSUB // 2, 1, hint_engines=(mybir.EngineType.PE,)
            ) as it:
                for j in range(2):
                    # steps of sub-chunk (2*it + j) consume gxb[j]
                    for s in range(TC):
                        step(j, s)
                    # project the next sub-chunk into the other buffer
                    gx_chunk(1 - j, it * (2 * TC * BL) + (j + 1) * TC * BL)
                    nc.sync.dma_start(
                        out=yt[:, bass.ds(it * (2 * TC * W) + j * TC * W, TC * W)],
                        in_=yac[j][:],
                    )

            nc.sync.dma_start(out=hc[:, 0:W], in_=yac[1][:, TC - 1, :])
            nc.sync.dma_start(out=hc[:, W : 2 * W], in_=c32[:])

    _split_excess_waits(nc)
    return nc


def _prep_core(inputs: dict, core: int) -> dict:
    d = core // 4  # 0 fwd, 1 rev
    b0 = (core % 4) * BL
    sfx = "f" if d == 0 else "r"
    Whh = np.asarray(inputs[f"Whh_{sfx}"])[_PERM]
    Wih = np.asarray(inputs[f"Wih_{sfx}"])[_PERM]
    bvec = (np.asarray(inputs[f"bih_{sfx}"]) + np.asarray(inputs[f"bhh_{sfx}"]))[_PERM]

    x = np.asarray(inputs["inputs"])[:, b0 : b0 + BL, :]  # [T, BL, D]
    if d == 1:
        x = x[::-1]
    xt = np.ascontiguousarray(x.transpose(2, 0, 1).reshape(D, T * BL))
    xt = np.concatenate([xt, np.zeros((D, TC * BL), xt.dtype)], axis=1)

    def hc_pack(a):  # [BL, H] -> [128, 2*BL] with col k*BL+b = a[b, k*128+p]
        return np.ascontiguousarray(
            a.T.reshape(2, 128, BL).transpose(1, 0, 2).reshape(128, 2 * BL)
        ).astype(np.float32)

    return {
        "xt": xt.astype(RNN_NP),
        "whh": np.ascontiguousarray(
            Whh.T.reshape(2, 128, G4).transpose(1, 0, 2).reshape(128, 2 * G4)
        ).astype(RNN_NP),
        "wih": np.ascontiguousarray(Wih.T).astype(RNN_NP),
        "bias": np.ascontiguousarray(bvec.reshape(8, 128).T).astype(np.float32),
        "h0": hc_pack(np.asarray(inputs["hx"])[d, b0 : b0 + BL]),
        "c0": hc_pack(np.asarray(inputs["cx"])[d, b0 : b0 + BL]),
    }


def prep_in_maps(inputs: dict) -> list[dict]:
    return [_prep_core(inputs, c) for c in range(NCORES)]


def postprocess(results: list[dict]):
    out = np.empty((T, B, 2 * H), np.float32)
    hy = np.empty((2, B, H), np.float32)
    cy = np.empty((2, B, H), np.float32)
    for core in range(NCORES):
        d = core // 4
        b0 = (core % 4) * BL
        # yt cols: t*32 + k*16 + b  ->  [T, BL, H] with h = k*128 + p
        y = (
            results[core]["yt"]
            .reshape(128, T, 2, BL)
            .transpose(1, 3, 2, 0)
            .reshape(T, BL, H)
        )
        if d == 1:
            y = y[::-1]
        out[:, b0 : b0 + BL, d * H : (d + 1) * H] = y

        hcv = results[core]["hc"]  # [128, 4*BL]: h then c

        def unpack(a):  # [128, 2*BL] -> [BL, H]
            return a.reshape(128, 2, BL).transpose(2, 1, 0).reshape(BL, H)

        hy[d, b0 : b0 + BL] = unpack(hcv[:, : 2 * BL])
        cy[d, b0 : b0 + BL] = unpack(hcv[:, 2 * BL :])
    return out, hy, cy


def kernel(**inputs):
    from concourse.bass_utils import run_bass_kernel_spmd

    nc = build_nc()
    in_maps = prep_in_maps(inputs)
    res = run_bass_kernel_spmd(nc, in_maps, list(range(NCORES)))
    return postprocess(res.results)


# revision 15
# speedup vs baseline: 43.2014x; 43.2014x over previous
"""BiLSTM layer (T=1024, B=64, D=128, H=256) on 8 TRN2 NeuronCores.

Sharding: direction x batch. Cores 0-3 run the forward LSTM, cores 4-7 the
reverse LSTM (fed time-flipped inputs so the compiled SPMD program is
identical on every core); each core owns a 16-batch slice.

Per-core layout ("transposed" gates): gates/h/c live as [feature-partition,
batch-free] tiles, so the per-step recurrent matmul is 16 LDWEIGHTS+MATMUL
pairs (8 gate tiles x 2 K-tiles) with Whh stationary in fp16 (fast weight
load) and hT as a 16-column moving operand. The input projection
gx = x @ Wih.T + b is computed on-device in 32-step chunks, double-buffered
and interleaved with the recurrence so it hides in PE gaps.

Gate order is host-permuted from torch's (i,f,g,o) to (g,i,f,o) so tanh gates
occupy one contiguous column block and sigmoid gates another.
"""

import numpy as np

import concourse.bass as bass
import concourse.mybir as mybir
from concourse.tile import TileContext

T, B, D, H = 1024, 64, 128, 256
G4 = 4 * H  # 1024
NCORES = 8
BL = B // 4  # 16 batches per core (4 cores per direction)
TC = 32  # time-steps per sub-chunk (=> 512 x-columns per gx matmul)
NSUB = T // TC  # 32 sub-chunks
F32 = mybir.dt.float32

# fp16 for the matmul operands (weights, h, x): fast weight load + 11-bit
# mantissa; accumulation stays fp32 in PSUM and all state/outputs are fp32.
RNN_DT = mybir.dt.float16
RNN_NP = np.float16

# row permutation taking torch gate order (i,f,g,o) -> (g,i,f,o)
_PERM = np.concatenate(
    [np.arange(512, 768), np.arange(0, 256), np.arange(256, 512), np.arange(768, 1024)]
)
# column offsets (units of BL) of each gate in the packed [128, 8*16] tile
_GC, _IC, _FC, _OC = 0, 2, 4, 6


def _split_excess_waits(nc, max_waits: int = 1):
    """This walrus build rejects instructions carrying more than one sync
    wait. Hoist excess waits onto same-engine nop instructions inserted
    immediately before the offending instruction (same program order on that
    engine's stream, so semantics are preserved)."""
    fn = nc.m.functions[0]
    # pass 1: find spills
    spills = []  # (bb_idx, inst_name, engine, [extra waits])
    for bi, bb in enumerate(fn.blocks):
        for ins in bb.instructions:
            si = ins.sync_info
            if si is None:
                continue
            waits = list(si.on_wait)
            if len(waits) > max_waits:
                spills.append((bi, str(ins.name), ins.engine, waits))
    if not spills:
        return
    # pass 2: create the nop instructions (they land appended to the current
    # tail block; pass 3 relocates them)
    nop_names = set()
    nops_for = {}
    for bi, iname, engine, waits in spills:
        nops = []
        for w in waits[max_waits:]:
            nop = nc.engines[engine].nop(nofuse=True)
            nop.ins.sync_info = mybir.SyncInfo(on_wait=[w], on_update=[])
            nop_names.add(str(nop.ins.name))
            nops.append(nop.ins)
        nops_for[(bi, iname)] = nops
    # pass 3: rebuild blocks with nops in position (and stripped from tail)
    new_blocks = []
    for bi, bb in enumerate(fn.blocks):
        out = []
        for ins in bb.instructions:
            nm = str(ins.name)
            if nm in nop_names:
                continue  # relocated
            pre = nops_for.get((bi, nm))
            if pre is not None:
                si = ins.sync_info
                ins.sync_info = mybir.SyncInfo(
                    on_wait=list(si.on_wait)[:max_waits],
                    on_update=list(si.on_update),
                )
                out.extend(pre)
            out.append(ins)
        new_blocks.append(
            mybir.BasicBlock(
                name=bb.name,
                instructions=out,
                IsPredicated=bb.IsPredicated,
                IsExit=bb.IsExit,
                IsLoopEntry=bb.IsLoopEntry,
            )
        )
    fn.blocks = new_blocks


def build_nc(nsub: int = NSUB, repeat: int = 1, variant: str = 'full', nm: int = 8) -> bass.Bass:
    """repeat>1 re-runs the whole recurrence loop (timing variants only)."""
    nc = bass.Bass()

    xt = nc.declare_dram_parameter("xt", [128, (T + TC) * BL], RNN_DT, isOutput=False)
    whh = nc.declare_dram_parameter("whh", [128, 2 * G4], RNN_DT, isOutput=False)
    wih = nc.declare_dram_parameter("wih", [128, G4], RNN_DT, isOutput=False)
    bias = nc.declare_dram_parameter("bias", [128, 8], F32, isOutput=False)
    h0 = nc.declare_dram_parameter("h0", [128, 2 * BL], F32, isOutput=False)
    c0 = nc.declare_dram_parameter("c0", [128, 2 * BL], F32, isOutput=False)
    yt = nc.declare_dram_parameter("yt", [128, T * 2 * BL], RNN_DT, isOutput=True)
    hc = nc.declare_dram_parameter("hc", [128, 4 * BL], F32, isOutput=True)

    W = 2 * BL  # 32: width of h/c tiles (2 K-tiles side by side)
    SIG = mybir.ActivationFunctionType.Sigmoid
    TANH = mybir.ActivationFunctionType.Tanh
    IDENT = mybir.ActivationFunctionType.Identity

    with TileContext(nc) as tc:
        with (
            tc.tile_pool(name="persist", bufs=1) as pp,
            tc.tile_pool(name="work", bufs=3) as wp,
            tc.tile_pool(name="pg", bufs=2, space="PSUM") as pgp,
            tc.tile_pool(name="pgx", bufs=3, space="PSUM") as pgxp,
        ):
            # ---- persistent tiles ----
            whh_sb = pp.tile([128, 2 * G4], RNN_DT, tag="whh_sb")
            wih_sb = pp.tile([128, G4], RNN_DT, tag="wih_sb")
            bias_sb = pp.tile([128, 8], F32, tag="bias_sb")
            c32 = pp.tile([128, W], F32, tag="c32")
            h0f = pp.tile([128, W], F32, tag="h0f")
            xc = [pp.tile([128, TC * BL], RNN_DT, tag=f"xc{j}", name=f"xc{j}") for j in range(2)]
            gxb = [pp.tile([128, 8, TC * BL], F32, tag=f"gxb{j}", name=f"gxb{j}") for j in range(2)]
            # y accumulator doubles as the h state: the recurrent matmul
            # reads h_{t-1} straight from the previous step's fp16 y slice
            yac = [pp.tile([128, TC, W], RNN_DT, tag=f"yac{j}", name=f"yac{j}") for j in range(2)]

            nc.sync.dma_start(out=whh_sb[:], in_=whh[:])
            nc.sync.dma_start(out=wih_sb[:], in_=wih[:])
            nc.sync.dma_start(out=bias_sb[:], in_=bias[:])
            nc.sync.dma_start(out=h0f[:], in_=h0[:])
            nc.sync.dma_start(out=c32[:], in_=c0[:])
            # seed h_{-1} where step (j=0, s=0) looks for it
            nc.vector.tensor_copy(yac[1][:, TC - 1, :], h0f[:])

            def gx_chunk(j: int, col0):
                """Project x columns [col0, col0+TC*BL) into gxb[j] (+bias)."""
                nc.sync.dma_start(out=xc[j][:], in_=xt[:, bass.ds(col0, TC * BL)])
                for m in range(8):
                    pgx = pgxp.tile([128, TC * BL], F32, tag="pgx")
                    nc.tensor.matmul(
                        pgx[:],
                        wih_sb[:, m * 128 : (m + 1) * 128],
                        xc[j][:],
                        start=True,
                        stop=True,
                    )
                    nc.scalar.activation(
                        gxb[j][:, m, :], pgx[:], IDENT, bias=bias_sb[:, m : m + 1]
                    )

            def h_prev(j: int, s: int):
                return yac[j][:, s - 1, :] if s > 0 else yac[1 - j][:, TC - 1, :]

            def step(j: int, s: int):
                """One LSTM time-step; gates from gxb[j][:, :, s*BL:(s+1)*BL]."""
                pg = pgp.tile([128, 8 * BL], F32, tag="pg")
                pg3 = pg.rearrange("p (m b) -> p m b", b=BL)
                hp = h_prev(j, s)
                for m in range(nm):
                    for k in range(2):
                        nc.tensor.matmul(
                            pg[:, m * BL : (m + 1) * BL],
                            whh_sb[:, k * G4 + m * 128 : k * G4 + (m + 1) * 128],
                            hp[:, k * BL : (k + 1) * BL],
                            start=(k == 0),
                            stop=(k == 1),
                        )
                if variant == "mmonly":
                    # timing probe: shortest possible tail (1 DVE hop)
                    nc.vector.tensor_copy(yac[j][:, s, :], pg[:, 0:W])
                    return
                gt = wp.tile([128, 8 * BL], F32, tag="gt")
                gt3 = gt.rearrange("p (m b) -> p m b", b=BL)
                gxs = gxb[j][:, :, s * BL : (s + 1) * BL]
                g_, i_, f_, o_ = (
                    slice(_GC * BL, (_GC + 2) * BL),
                    slice(_IC * BL, (_IC + 2) * BL),
                    slice(_FC * BL, (_FC + 2) * BL),
                    slice(_OC * BL, (_OC + 2) * BL),
                )
                nc.vector.tensor_add(gt3[:, :, :], pg3[:, :, :], gxs[:, :, :])
                ac = wp.tile([128, 8 * BL], F32, tag="ac")
                nc.scalar.activation(ac[:, g_], gt[:, g_], TANH)
                nc.scalar.activation(
                    ac[:, _IC * BL : (_OC + 2) * BL], gt[:, _IC * BL : (_OC + 2) * BL], SIG
                )
                t1 = wp.tile([128, W], F32, tag="t1")
                t2 = wp.tile([128, W], F32, tag="t2")
                nc.vector.tensor_mul(t1[:], ac[:, i_], ac[:, g_])
                nc.vector.tensor_mul(t2[:], ac[:, f_], c32[:])
                nc.vector.tensor_add(c32[:], t1[:], t2[:])
                tcc = wp.tile([128, W], F32, tag="tcc")
                nc.scalar.activation(tcc[:], c32[:], TANH)
                nc.vector.tensor_mul(yac[j][:, s, :], ac[:, o_], tcc[:])

            # prologue: gx for sub-chunk 0 into buffer 0
            gx_chunk(0, 0)

            def loop_body(it):
                for j in range(2):
                    # steps of sub-chunk (2*it + j) consume gxb[j]
                    for s in range(TC):
                        step(j, s)
                    # project the next sub-chunk into the other buffer
                    gx_chunk(1 - j, it * (2 * TC * BL) + (j + 1) * TC * BL)
                    nc.sync.dma_start(
                        out=yt[:, bass.ds(it * (2 * TC * W) + j * TC * W, TC * W)],
                        in_=yac[j][:],
                    )

            if repeat == 1:
                with tc.For_i(
                    0, nsub // 2, 1, hint_engines=(mybir.EngineType.PE,)
                ) as it:
                    loop_body(it)
            else:
                with tc.For_i(0, repeat, 1) as _rep:
                    with tc.For_i(
                        0, nsub // 2, 1, hint_engines=(mybir.EngineType.PE,)
                    ) as it:
                        loop_body(it)

            # gpsimd DMA casts the fp16 h state up to the fp32 output
            nc.gpsimd.dma_start(out=hc[:, 0:W], in_=yac[1][:, TC - 1, :])
            nc.sync.dma_start(out=hc[:, W : 2 * W], in_=c32[:])

    _split_excess_waits(nc)
    return nc


def _prep_core(inputs: dict, core: int) -> dict:
    d = core // 4  # 0 fwd, 1 rev
    b0 = (core % 4) * BL
    sfx = "f" if d == 0 else "r"
    Whh = np.asarray(inputs[f"Whh_{sfx}"])[_PERM]
    Wih = np.asarray(inputs[f"Wih_{sfx}"])[_PERM]
    bvec = (np.asarray(inputs[f"bih_{sfx}"]) + np.asarray(inputs[f"bhh_{sfx}"]))[_PERM]

    x = np.asarray(inputs["inputs"])[:, b0 : b0 + BL, :]  # [T, BL, D]
    if d == 1:
        x = x[::-1]
    xt = np.ascontiguousarray(x.transpose(2, 0, 1).reshape(D, T * BL))
    xt = np.concatenate([xt, np.zeros((D, TC * BL), xt.dtype)], axis=1)

    def hc_pack(a):  # [BL, H] -> [128, 2*BL] with col k*BL+b = a[b, k*128+p]
        return np.ascontiguousarray(
            a.T.reshape(2, 128, BL).transpose(1, 0, 2).reshape(128, 2 * BL)
        ).astype(np.float32)

    return {
        "xt": xt.astype(RNN_NP),
        "whh": np.ascontiguousarray(
            Whh.T.reshape(2, 128, G4).transpose(1, 0, 2).reshape(128, 2 * G4)
        ).astype(RNN_NP),
        "wih": np.ascontiguousarray(Wih.T).astype(RNN_NP),
        "bias": np.ascontiguousarray(bvec.reshape(8, 128).T).astype(np.float32),
        "h0": hc_pack(np.asarray(inputs["hx"])[d, b0 : b0 + BL]),
        "c0": hc_pack(np.asarray(inputs["cx"])[d, b0 : b0 + BL]),
    }


def prep_in_maps(inputs: dict) -> list[dict]:
    return [_prep_core(inputs, c) for c in range(NCORES)]


def postprocess(results: list[dict]):
    out = np.empty((T, B, 2 * H), np.float32)
    hy = np.empty((2, B, H), np.float32)
    cy = np.empty((2, B, H), np.float32)
    for core in range(NCORES):
        d = core // 4
        b0 = (core % 4) * BL
        # yt cols: t*32 + k*16 + b  ->  [T, BL, H] with h = k*128 + p
        y = (
            results[core]["yt"]
            .astype(np.float32)
            .reshape(128, T, 2, BL)
            .transpose(1, 3, 2, 0)
            .reshape(T, BL, H)
        )
        if d == 1:
            y = y[::-1]
        out[:, b0 : b0 + BL, d * H : (d + 1) * H] = y

        hcv = results[core]["hc"]  # [128, 4*BL]: h then c

        def unpack(a):  # [128, 2*BL] -> [BL, H]
            return a.reshape(128, 2, BL).transpose(2, 1, 0).reshape(BL, H)

        hy[d, b0 : b0 + BL] = unpack(hcv[:, : 2 * BL])
        cy[d, b0 : b0 + BL] = unpack(hcv[:, 2 * BL :])
    return out, hy, cy


def kernel(**inputs):
    from concourse.bass_utils import run_bass_kernel_spmd

    nc = build_nc()
    in_maps = prep_in_maps(inputs)
    res = run_bass_kernel_spmd(nc, in_maps, list(range(NCORES)))
    return postprocess(res.results)
